# revision 3
# baseline (speedup 1.0000x reference)
"""Trainium2 Bass kernel: 24x24-bit array multiplier (bit-vector in/out).

Inputs  A, B: [131072, 24] f32 {0,1} bits, LSB-first.
Output: [131072, 48] f32 {0,1} product bits, LSB-first.

Pure data-parallel across 8 NeuronCores (16384 rows each). Per core,
per chunk of rows, with A and B halves packed in ONE tile so every
stage is a single instruction over both operands:

  1. (optional, COMBINE_LEVELS) scalar_tensor_tensor passes (mult/add)
     fold bit pairs into base-4/base-16 digits (digit = base*odd+even).
  2. One Horner scan (tensor_tensor_scan, multiplier 1/base, reset 0
     at group starts) turns bits/digits into scaled 12-bit limbs.
  3. One limb gather (tensor_scalar mult at the DVE 2x_2p rate, or
     ScalarE activation Copy), f32->int32 -> [a0|a1|b0|b1] limb-major.
  4. Two scalar_tensor_tensor (bypass, mult) products ->
     pt = [p00|p10|p01|p11], then an 8-instruction exact base-4096
     carry chain that overwrites pt in place to [t0|t1|t2|t3]:
       t1 = p01 + (p10 & fff) + (p00 >> 12)   (fits < 2^24 exactly)
       t2 = p11 + (p10 >> 12) + (t1 >> 12)    (fits < 2^24 exactly)
       t3 = t2 >> 12
     Digits keep junk bits >= 12; extraction never reads them. All
     values < 2^24 so DVE int32 (fp32-internal) math is exact.
  5. Bit extraction: 12 single-op tensor_scalar ANDs (digit & (1<<s))
     at the DVE 2x_2p rate write int32 {0, 2^s} in DRAM row-major
     order; ScalarE Sign converts to f32 {0,1} per piece; per-piece
     output DMA. Input DMAs are all prefetched on SP so no output wait
     blocks them.

Constants (scan multiplier pattern) are built on the otherwise-idle
Pool/GPSIMD engine during the first input DMA.

All arithmetic is exact; rel err vs reference = 0.
"""

import numpy as np

import concourse.bass as bass
import concourse.bacc as bacc
import concourse.mybir as mybir
from concourse.bass_utils import run_bass_kernel_spmd
from concourse.tile import TileContext

F32 = mybir.dt.float32
I32 = mybir.dt.int32
OP = mybir.AluOpType
AF = mybir.ActivationFunctionType

P = 128            # SBUF partitions
N_CORES = 8
N_ROWS = 131072    # total batch
N_SHARD = N_ROWS // N_CORES  # 16384 rows per core

CHUNKS = [24, 40, 40, 24]  # rows/partition per chunk; sum must be 128
COMBINE_LEVELS = 0      # 0: scan raw bits; 1: base-4 digits; 2: base-16 digits
P02_ON_POOL = True      # build scan-multiplier constant on GPSIMD
GATHER_ON_ACT = False   # limb gather on ScalarE (else DVE tensor_scalar)
PREFETCH_ALL = True     # issue every input DMA before any output DMA
B_FIRST = False         # DMA the B half before the A half
SIGN_PIECES = 2         # Sign+out-DMA pieces per chunk
LAST_SIGN_ON_DVE = True  # last piece: is_gt on DVE instead of ACT Sign
XT_BUFS = 3             # buffers for the extraction staging pool
EXTR_PRIO = 0           # high_priority offset for extraction+sign+dma (0 = off)
SPLIT_FIRST = True      # chunk 0: scan A and B separately (starts earlier)
STAGGER_US = 4.5        # tile_wait_until stagger per chunk for scans (µs)
EXTR_FENCE = False      # order scan k+1 after extraction k via a WAW fence


def _build_nc(n_rows: int, chunks=None, repeats: int = 1) -> bass.Bass:
    R = n_rows // P          # rows per partition
    chunks = chunks or CHUNKS
    assert sum(chunks) == R, (chunks, R)
    Rmax = max(chunks)

    nc = bacc.Bacc()
    A = nc.declare_dram_parameter("A", [n_rows, 24], F32, isOutput=False)
    B = nc.declare_dram_parameter("B", [n_rows, 24], F32, isOutput=False)
    O = nc.declare_dram_parameter("out", [n_rows, 48], F32, isOutput=True)

    # partition p <-> DRAM rows [p*R, (p+1)*R); contiguous per partition
    Av = A[:].rearrange("(p r) b -> p r b", p=P)
    Bv = B[:].rearrange("(p r) b -> p r b", p=P)
    Ov = O[:].rearrange("(p r) b -> p r b", p=P)

    with TileContext(nc) as tc:
        with (
            tc.tile_pool(name="const", bufs=1) as cpool,
            tc.tile_pool(name="io", bufs=max(3, len(chunks))) as iopool,
            tc.tile_pool(name="work", bufs=3) as wpool,
            tc.tile_pool(name="ss", bufs=1) as sspool,
            tc.tile_pool(name="xt", bufs=XT_BUFS) as xtpool,
            tc.tile_pool(
                name="ot",
                bufs=min(
                    len(chunks) * SIGN_PIECES + 1,
                    max(3, 36864 // ((Rmax // SIGN_PIECES) * 48 * 4)),
                ),
            ) as otpool,
        ):
            LV = COMBINE_LEVELS
            GE = 12 >> LV                       # scan group length
            SW = 24 >> LV                       # scan elems per row
            MULT = 1.0 / (1 << (1 << LV))       # 0.5 / 0.25 / 0.0625
            GSCALE = float((1 << (1 << LV)) ** (GE - 1))
            eng0 = nc.gpsimd if P02_ON_POOL else nc.vector
            p02 = cpool.tile([P, 2 * Rmax * SW], F32, tag="p02")
            eng0.memset(p02[:], MULT)
            eng0.memset(
                p02[:].rearrange("p (g e) -> p g e", e=GE)[:, :, 0:1], 0.0
            )

            # up-front input DMAs (SP sequencer never blocks on compute)
            ab_tiles = []
            c0 = 0
            for ci, Rc in enumerate(chunks):
                ab_t = iopool.tile([P, 2 * Rc * 24], F32, tag=f"ab{ci}")
                halves = [
                    (ab_t[:, 0 : Rc * 24], Av),
                    (ab_t[:, Rc * 24 : 2 * Rc * 24], Bv),
                ]
                if B_FIRST:
                    halves.reverse()
                for dst, srcv in halves:
                    nc.sync.dma_start(out=dst, in_=srcv[:, c0 : c0 + Rc, :])
                ab_tiles.append(ab_t)
                c0 += Rc

            ss_tiles = [
                sspool.tile(
                    [P, 2 * Rc * SW], F32, tag=f"ss{ci}", name=f"sst{ci}"
                )
                for ci, Rc in enumerate(chunks)
            ]

            c0 = 0
            for ci, Rc in enumerate(chunks):
                last_chunk = ci == len(chunks) - 1
                ab_t = ab_tiles[ci]

                # 1+2. combines then one Horner scan over both halves
                x_in = ab_t
                for lv in range(LV):
                    w = 2 * Rc * (24 >> (lv + 1))
                    base = float(1 << (1 << lv))
                    cc = wpool.tile([P, w], F32, tag=f"cc{lv}")
                    v2 = x_in[:].rearrange("p (j two) -> p j two", two=2)
                    nc.vector.scalar_tensor_tensor(
                        cc[:].unsqueeze(2), v2[:, :, 1:2], base,
                        v2[:, :, 0:1], OP.mult, OP.add,
                    )
                    x_in = cc
                ss = ss_tiles[ci]
                if STAGGER_US:
                    tc.tile_set_cur_wait(ci * STAGGER_US / 1000.0)
                if SPLIT_FIRST and ci == 0:
                    # separate scans per operand half: the first starts as
                    # soon as its own DMA lands
                    hw_ = Rc * SW
                    for off in (0, hw_):
                        nc.vector.tensor_tensor_scan(
                            ss[:, off : off + hw_], p02[:, :hw_],
                            x_in[:, off : off + hw_], 0.0, OP.mult, OP.add,
                        )
                else:
                    nc.vector.tensor_tensor_scan(
                        ss[:], p02[:, : 2 * Rc * SW], x_in[:], 0.0,
                        OP.mult, OP.add,
                    )

                # 3. limb gather -> li = [a0|a1|b0|b1], int32 limb-major
                li = wpool.tile([P, 4 * Rc], I32, tag="li")
                sv = ss[:].rearrange(
                    "p (h r l e) -> p h l r e", h=2, l=2, e=GE
                )[:, :, :, :, GE - 1]
                dv = li[:].rearrange("p (h l r) -> p h l r", h=2, l=2)
                if GATHER_ON_ACT and not last_chunk:
                    nc.scalar.activation(dv, sv, AF.Copy, scale=GSCALE)
                else:
                    nc.vector.tensor_scalar(dv, sv, GSCALE, None, OP.mult)

                # 4. products pt = [p00|p10|p01|p11] then in-place carry
                # chain -> [t0|t1|t2|t3]
                pt = wpool.tile([P, 4 * Rc], I32, tag="pt")
                pv = pt[:].rearrange("p (k r) -> p k r", k=4)
                aa = li[:, 0 : 2 * Rc].rearrange("p (l r) -> p l r", l=2)
                b0 = li[:, 2 * Rc : 3 * Rc].unsqueeze(1)
                b1 = li[:, 3 * Rc : 4 * Rc].unsqueeze(1)
                nc.vector.scalar_tensor_tensor(
                    pv[:, 0:2, :], aa, 0, b0.broadcast_to([P, 2, Rc]),
                    OP.bypass, OP.mult,
                )
                nc.vector.scalar_tensor_tensor(
                    pv[:, 2:4, :], aa, 0, b1.broadcast_to([P, 2, Rc]),
                    OP.bypass, OP.mult,
                )
                sc = wpool.tile([P, 4 * Rc], I32, tag="sc")
                kh = sc[:, 0 : 2 * Rc].rearrange("p (k r) -> p k r", k=2)
                k0 = kh[:, 0:1, :]
                h10 = kh[:, 1:2, :]
                l10 = sc[:, 2 * Rc : 3 * Rc].unsqueeze(1)
                m1 = sc[:, 3 * Rc : 4 * Rc].unsqueeze(1)
                # [k0, h10] = [p00, p10] >> 12 ; l10 = p10 & fff
                nc.vector.tensor_scalar(
                    kh, pv[:, 0:2, :], 12, None, OP.arith_shift_right
                )
                nc.vector.tensor_scalar(
                    l10, pv[:, 1:2, :], 4095, None, OP.bitwise_and
                )
                # t1 = p01 + (l10 + k0)   (slot1; p10 dead)
                nc.vector.scalar_tensor_tensor(
                    m1, l10, 0, k0, OP.bypass, OP.add
                )
                nc.vector.scalar_tensor_tensor(
                    pv[:, 1:2, :], pv[:, 2:3, :], 0, m1, OP.bypass, OP.add
                )
                # t2 = p11 + (h10 + (t1 >> 12))   (slot2; p01 dead)
                nc.vector.tensor_scalar(
                    l10, pv[:, 1:2, :], 12, None, OP.arith_shift_right
                )
                nc.vector.scalar_tensor_tensor(
                    m1, h10, 0, l10, OP.bypass, OP.add
                )
                nc.vector.scalar_tensor_tensor(
                    pv[:, 2:3, :], pv[:, 3:4, :], 0, m1, OP.bypass, OP.add
                )
                # t3 = t2 >> 12   (slot3; p11 dead)
                nc.vector.tensor_scalar(
                    pv[:, 3:4, :], pv[:, 2:3, :], 12, None,
                    OP.arith_shift_right,
                )

                # 5. extraction: 12 single-op ANDs -> int32 {0, 2^s} in
                # DRAM row-major order; Sign per piece -> f32; per-piece
                # output DMA.
                import contextlib
                prio_ctx = (
                    tc.high_priority(offset=EXTR_PRIO)
                    if EXTR_PRIO else contextlib.nullcontext()
                )
                with prio_ctx:
                    xt = xtpool.tile([P, Rc * 48], I32, tag="x")
                    d4 = pt[:].rearrange("p (k r) -> p r k", k=4)
                    xv4 = xt[:].rearrange("p (r k s) -> p r k s", k=4, s=12)
                    for s in range(12):
                        nc.vector.tensor_scalar(
                            xv4[:, :, :, s], d4, 1 << s, None, OP.bitwise_and
                        )
                    if EXTR_FENCE and not last_chunk:
                        # WAW fence: next chunk's scan overwrites this 1-elem
                        # write, ordering it after this chunk's extraction
                        nc.vector.tensor_scalar(
                            ss_tiles[ci + 1][:, 0:1],
                            xt[:, Rc * 48 - 1 : Rc * 48],
                            0, None, OP.bitwise_or,
                        )
                    npc = SIGN_PIECES if Rc % SIGN_PIECES == 0 else 1
                    Rh = Rc // npc
                    for h in range(npc):
                        o_t = otpool.tile([P, Rh * 48], F32, tag="o")
                        xs = xt[:, h * Rh * 48 : (h + 1) * Rh * 48]
                        last_piece = last_chunk and h == npc - 1
                        if LAST_SIGN_ON_DVE and last_piece:
                            nc.vector.tensor_scalar(
                                o_t[:], xs, 0, None, OP.is_gt
                            )
                        else:
                            nc.scalar.activation(o_t[:], xs, AF.Sign)
                        nc.sync.dma_start(
                            out=Ov[:, c0 + h * Rh : c0 + (h + 1) * Rh, :],
                            in_=o_t[:],
                        )
                c0 += Rc

    nc.finalize()
    return nc


_CACHE = {}


def _get_nc():
    key = (N_SHARD, tuple(CHUNKS))
    if key not in _CACHE:
        _CACHE[key] = _build_nc(N_SHARD)
    return _CACHE[key]


def kernel(A: np.ndarray, B: np.ndarray) -> np.ndarray:
    A = np.ascontiguousarray(A, dtype=np.float32)
    B = np.ascontiguousarray(B, dtype=np.float32)
    nc = _get_nc()
    in_maps = [
        {
            "A": A[c * N_SHARD : (c + 1) * N_SHARD],
            "B": B[c * N_SHARD : (c + 1) * N_SHARD],
        }
        for c in range(N_CORES)
    ]
    res = run_bass_kernel_spmd(nc, in_maps, core_ids=list(range(N_CORES)))
    return np.concatenate([res.results[i]["out"] for i in range(N_CORES)], axis=0)
